# revision 6
# baseline (speedup 1.0000x reference)
"""Trainium2 Bass kernel for nn_CAGKE_1 (Gaussian-kernel embedding).

Math: reference computes, for mask m_i = 1[X_i > 0.5],
    out[j] = sum_e softmax(w)_e * sum_i m_i * (c/sigma_e) exp(-(j-i-1)^2/(2 sigma_e^2)) + noise_j
Both sums are linear, so the E=128 Gaussian channels collapse into one
combined kernel ghat(d) = sum_e softmax(w)_e * (c/sigma_e) exp(-d^2/(2 sigma_e^2))
BEFORE the convolution. sigma_max = 5 makes ghat(d) < 1e-9 for |d| >= 32,
so 64 taps (d = 31-u, u in [0,64)) cover it to far below the accuracy gate.

Layout strategy (per core, 1024 outputs):
  - The conv is ONE matmul shape: out[n] = sum_u gs[u] * M[u, n] with
    M[u, n] = m[base + u + n].  M is materialized by a single DMA whose
    DRAM access pattern has stride 1 in BOTH dims over a 1088-byte padded
    mask window - the 64x Toeplitz expansion happens inside the DMA
    (64 descriptors of 1KB), so the host ships 1.1KB instead of 144KB.
  - sigma/w arrive as ONE 256-wide f32 row on partition 0 (1 descriptor);
    the whole softmax/sigma chain runs in row space on partition 0, then
    two tiny PE transposes (moving = const [1,1] ones) lift
    t1 = c*exp(w)/(sigma*Z) and ivs = +1/(2 sigma^2) into column space.
    Z comes free from the Exp activation's accum_out, so there is no
    fp32 ones-matmul anywhere.  The exp table gets a NEGATED d^2 so ivs
    needs no sign flip.
  - The output lives on PSUM partitions {0,32,64,96} x 256 cols: four
    col-tiled matmuls (tile_position=(0,32k)) with the same 1-column gs
    stationary run CONCURRENTLY on the PE sub-arrays, each streaming a
    shifted 256-col slice of M.  A sparse-stationary matmul (S4[k,32k]=1)
    pre-loads the noise into the same PSUM region (start=True), so the
    epilogue is one DVE copy + a 4-descriptor store.
  - The X-0.5 shift is scaled by 64 on the host (pure affine + fp8 cast,
    as in the reference's binarize-vs-0.5) so the sign survives fp8;
    the device binarizes with is_gt 0.
"""

import sys

import numpy as np

if "/opt/trn_rl_repo" not in sys.path:
    sys.path.insert(0, "/opt/trn_rl_repo")

T = 8192
E = 128
N_CORES = 8
TJ = T // N_CORES          # 1024 outputs per core
NTAP = 64                  # ghat taps: d = 31 - u, u in [0, 64)
WIN = NTAP + TJ            # 1088-byte padded mask window per core
INV_SQRT_2PI = 0.39894228

_compiled = None


def _build():
    import concourse.bacc as bacc
    import concourse.bass as bass
    import concourse.mybir as mybir
    import concourse.tile as tile
    from concourse.ap import AP

    f32 = mybir.dt.float32
    bf16 = mybir.dt.bfloat16
    fp8 = mybir.dt.float8e4
    nc = bacc.Bacc(num_devices=N_CORES, debug=False)

    crit_d = nc.dram_tensor("crit", [1, 256], f32, kind="ExternalInput")
    maskr_d = nc.dram_tensor("maskr", [1, WIN], fp8, kind="ExternalInput")
    noise_d = nc.dram_tensor("noisem", [4, 256], bf16, kind="ExternalInput")
    out_d = nc.dram_tensor("out", [4, 256], f32, kind="ExternalOutput")

    # constants baked into the NEFF (loaded to HBM at model-load time)
    import ml_dtypes

    s4_np = np.zeros((4, 128), dtype=np.float32)
    for k in range(4):
        s4_np[k, 32 * k] = 1.0
    one1_d = nc.inline_tensor(np.ones((1, 1), dtype=np.float32), name="one1c")
    s4_d = nc.inline_tensor(s4_np.astype(ml_dtypes.bfloat16), name="s4c")

    with tile.TileContext(nc) as tc:
        with (
            tc.tile_pool(name="pool", bufs=1) as pool,
            tc.tile_pool(name="psum", bufs=1, space="PSUM") as psum,
        ):
            # ---- input + const loads; critical (sigma/w) first ----
            crit = pool.tile([1, 256], f32, tag="crit")
            nc.sync.dma_start(crit[:], crit_d[:])
            one1 = pool.tile([1, 1], f32, tag="one1")
            nc.sync.dma_start(one1[:], one1_d[:])
            s4 = pool.tile([4, 128], bf16, tag="s4")
            nc.sync.dma_start(s4[:], s4_d[:])
            noisem = pool.tile([4, 256], bf16, tag="noisem")
            nc.sync.dma_start(noisem[:], noise_d[:])
            # Toeplitz-expanding mask load: partition u reads bytes
            # [u, u+1024) of the 1088-byte window (overlapping reads).
            mraw = pool.tile([NTAP, TJ], fp8, tag="mraw")
            mask_src = AP(maskr_d[:].tensor, 0, [(1, NTAP), (1, TJ)])
            nc.sync.dma_start(mraw[:], mask_src)

            wrow = crit[0:1, 0:128]
            srow = crit[0:1, 128:256]

            # ---- input-independent prep (Pool engine, off-path) ----
            # d2n[., u] = -(u-31)^2 via (u-31)*(31-u)
            dlt = pool.tile([128, NTAP], f32, tag="dlt")
            nc.gpsimd.iota(
                dlt[:], pattern=[[1, NTAP]], base=-31, channel_multiplier=0,
                allow_small_or_imprecise_dtypes=True,
            )
            dltn = pool.tile([128, NTAP], f32, tag="dltn")
            nc.gpsimd.iota(
                dltn[:], pattern=[[-1, NTAP]], base=31, channel_multiplier=0,
                allow_small_or_imprecise_dtypes=True,
            )
            d2n = pool.tile([128, NTAP], f32, tag="d2n")
            nc.gpsimd.tensor_mul(d2n[:], dlt[:], dltn[:])

            # ---- row-space softmax/sigma chain (all on partition 0) ----
            # rows1 cols 0:128 = t1 = c*exp(w)/(sigma*Z); cols 128:256 = 1/(2 sigma^2)
            rows1 = pool.tile([1, 256], f32, tag="rows1")
            t2 = pool.tile([1, 128], f32, tag="t2")
            nc.vector.tensor_mul(t2[:], srow, srow)
            s2 = pool.tile([1, 128], f32, tag="s2")
            nc.vector.tensor_scalar_mul(s2[:], t2[:], 2.0)
            nc.vector.reciprocal(rows1[:, 128:256], s2[:])
            rsr = pool.tile([1, 128], f32, tag="rsr")
            nc.vector.reciprocal(rsr[:], srow)
            ew = pool.tile([1, 132], f32, tag="ew")
            nc.scalar.activation(
                ew[:, 0:128], wrow, mybir.ActivationFunctionType.Exp,
                accum_out=ew[:, 128:129],
            )
            czr = pool.tile([1, 1], f32, tag="czr")
            nc.vector.reciprocal(czr[:], ew[:, 128:129])
            t1a = pool.tile([1, 128], f32, tag="t1a")
            nc.vector.tensor_mul(t1a[:], ew[:, 0:128], rsr[:])
            nc.vector.tensor_scalar(
                rows1[:, 0:128], t1a[:], czr[:], INV_SQRT_2PI,
                mybir.AluOpType.mult, mybir.AluOpType.mult,
            )

            # ---- two tiny PE transposes: row -> column ----
            pTa = psum.tile([128, 1], f32, tag="pTa")
            nc.tensor.matmul(pTa[:], rows1[:, 0:128], one1[:], is_transpose=True)
            pTb = psum.tile([128, 1], f32, tag="pTb")
            nc.tensor.matmul(pTb[:], rows1[:, 128:256], one1[:], is_transpose=True)
            ivc = pool.tile([128, 1], f32, tag="ivc")
            nc.vector.tensor_copy(ivc[:], pTb[:])
            t1c = pool.tile([128, 1], bf16, tag="t1c")
            nc.scalar.activation(
                t1c[:], pTa[:], mybir.ActivationFunctionType.Copy
            )

            # ---- per-sigma exp table and combined kernel gs ----
            expt = pool.tile([128, NTAP], bf16, tag="expt")
            nc.scalar.activation(
                expt[:], d2n[:], mybir.ActivationFunctionType.Exp, scale=ivc[:]
            )
            gp = psum.tile([64, 1], f32, tag="gp")
            nc.tensor.matmul(gp[:], expt[:], t1c[:], start=True, stop=True)
            gs = pool.tile([64, 1], bf16, tag="gs")
            nc.vector.tensor_copy(gs[:], gp[:])

            # ---- binarize the Toeplitz mask (X-0.5 > 0), 2 DVE chunks ----
            mb = pool.tile([NTAP, TJ], bf16, tag="mb")
            for lo, hi in ((0, 512), (512, TJ)):
                nc.vector.tensor_scalar(
                    mb[:, lo:hi], mraw[:, lo:hi], 0.0, None, mybir.AluOpType.is_gt
                )

            # ---- output PSUM: noise via sparse-stationary matmul, then
            #      4 concurrent col-tiled conv matmuls ----
            po = psum.tile([128, 256], f32, tag="po")
            # start+stop on the noise matmul opens and closes the sim's
            # group tracking in one go; the partial-partition conv matmuls
            # accumulate with the bank-granular group check bypassed
            # (stop is sim-only; HW accumulation is per-element has_written).
            nc.tensor.matmul(po[:], s4[:], noisem[:], start=True, stop=True)
            for k in range(4):
                nc.tensor.matmul(
                    po[32 * k : 32 * k + 1, :], gs[:], mb[:, 256 * k : 256 * (k + 1)],
                    start=False, stop=False,
                    tile_position=(0, 32 * k),
                    skip_group_check=True,
                )

            # ---- store: copy live PSUM partitions out, 4 descriptors ----
            outS = pool.tile([128, 256], f32, tag="outS")
            nc.vector.tensor_copy(outS[:], po[:])
            out_src = AP(outS[:].tensor, outS[:].offset, [(32 * 256, 4), (1, 256)])
            nc.sync.dma_start(out_d[:], out_src)

    nc.compile()
    return nc


def kernel(X, sigma, weight, noise):
    global _compiled
    import ml_dtypes

    from concourse.bass_utils import run_bass_kernel_spmd

    X = np.ascontiguousarray(np.asarray(X, dtype=np.float32)).reshape(1, T)
    sigma = np.ascontiguousarray(np.asarray(sigma, dtype=np.float32)).reshape(E)
    weight = np.ascontiguousarray(np.asarray(weight, dtype=np.float32)).reshape(1, E)
    noise = np.ascontiguousarray(np.asarray(noise, dtype=np.float32)).reshape(1, T)

    if _compiled is None:
        _compiled = _build()
    nc = _compiled

    # mask window: 64*(X-0.5) as fp8 (sign-preserving affine shift; the
    # device binarizes with >0).  Window for core c covers global indices
    # [c*1024 - 32, c*1024 + 1055]; out-of-range pads to -32 (mask 0).
    Xpad = np.full(T + NTAP, -32.0, dtype=np.float32)
    Xpad[NTAP // 2 : NTAP // 2 + T] = 64.0 * (X[0] - 0.5)
    in_maps = []
    for c in range(N_CORES):
        crit = np.empty((1, 256), dtype=np.float32)
        crit[0, 0:128] = weight[0]
        crit[0, 128:256] = sigma
        maskr = Xpad[c * TJ : c * TJ + WIN].astype(ml_dtypes.float8_e4m3)
        noisem = (
            noise[0, c * TJ : (c + 1) * TJ].reshape(4, 256).astype(ml_dtypes.bfloat16)
        )
        in_maps.append(
            {"crit": crit, "maskr": maskr.reshape(1, WIN), "noisem": noisem}
        )

    res = run_bass_kernel_spmd(nc, in_maps, core_ids=list(range(N_CORES)))
    out = np.empty((1, T), dtype=np.float32)
    for c in range(N_CORES):
        out[0, c * TJ : (c + 1) * TJ] = res.results[c]["out"].reshape(-1)
    return out


# revision 8
# speedup vs baseline: 1.1004x; 1.1004x over previous
"""Trainium2 Bass kernel for nn_CAGKE_1 (Gaussian-kernel embedding).

Math: reference computes, for mask m_i = 1[X_i > 0.5],
    out[j] = sum_e softmax(w)_e * sum_i m_i * (c/sigma_e) exp(-(j-i-1)^2/(2 sigma_e^2)) + noise_j
Both sums are linear, so the E=128 Gaussian channels collapse into one
combined kernel ghat(d) = sum_e softmax(w)_e * (c/sigma_e) exp(-d^2/(2 sigma_e^2))
BEFORE the convolution. sigma_max = 5 makes ghat(d) < 1e-9 for |d| >= 32,
so 64 taps (d = 31-u, u in [0,64)) cover it to far below the accuracy gate.

Layout strategy (per core, 1024 outputs):
  - The conv is ONE matmul shape: out[n] = sum_u gs[u] * M[u, n] with
    M[u, n] = m[base + u + n].  M is materialized by a single DMA whose
    DRAM access pattern has stride 1 in BOTH dims over a 1088-byte padded
    mask window - the 64x Toeplitz expansion happens inside the DMA
    (64 descriptors of 1KB), so the host ships 1.1KB instead of 144KB.
  - sigma/w arrive as ONE 256-wide f32 row on partition 0 (1 descriptor);
    the whole softmax/sigma chain runs in row space on partition 0, then
    two tiny PE transposes (moving = const [1,1] ones) lift
    t1 = c*exp(w)/(sigma*Z) and ivs = +1/(2 sigma^2) into column space.
    Z comes free from the Exp activation's accum_out, so there is no
    fp32 ones-matmul anywhere.  The exp table gets a NEGATED d^2 so ivs
    needs no sign flip.
  - The output lives on PSUM partitions {0,32,64,96} x 256 cols: four
    col-tiled matmuls (tile_position=(0,32k)) with the same 1-column gs
    stationary run CONCURRENTLY on the PE sub-arrays, each streaming a
    shifted 256-col slice of M.  A sparse-stationary matmul (S4[k,32k]=1)
    pre-loads the noise into the same PSUM region (start=True), so the
    epilogue is one DVE copy + a 4-descriptor store.
  - The X-0.5 shift is scaled by 64 on the host (pure affine + fp8 cast,
    as in the reference's binarize-vs-0.5) so the sign survives fp8;
    the device binarizes with is_gt 0.
"""

import sys

import numpy as np

if "/opt/trn_rl_repo" not in sys.path:
    sys.path.insert(0, "/opt/trn_rl_repo")

T = 8192
E = 128
N_CORES = 8
TJ = T // N_CORES          # 1024 outputs per core
NTAP = 64                  # ghat taps: d = 31 - u, u in [0, 64)
WIN = NTAP + TJ            # 1088-byte padded mask window per core
INV_SQRT_2PI = 0.39894228

_compiled = None


def _build():
    import concourse.bacc as bacc
    import concourse.bass as bass
    import concourse.mybir as mybir
    import concourse.tile as tile
    from concourse.ap import AP

    f32 = mybir.dt.float32
    bf16 = mybir.dt.bfloat16
    fp8 = mybir.dt.float8e4
    nc = bacc.Bacc(num_devices=N_CORES, debug=False)

    crit_d = nc.dram_tensor("crit", [1, 256], f32, kind="ExternalInput")
    maskr_d = nc.dram_tensor("maskr", [1, WIN], fp8, kind="ExternalInput")
    nm_d = nc.dram_tensor("nm", [4, 384], bf16, kind="ExternalInput")
    out_d = nc.dram_tensor("out", [4, 256], f32, kind="ExternalOutput")

    with tile.TileContext(nc) as tc:
        with (
            tc.tile_pool(name="pool", bufs=1) as pool,
            tc.tile_pool(name="psum", bufs=1, space="PSUM") as psum,
        ):
            # ---- three HWDGE loads; critical (sigma/w) first ----
            crit = pool.tile([1, 256], f32, tag="crit")
            nc.sync.dma_start(crit[:], crit_d[:])
            # Toeplitz-expanding mask load: partition u reads bytes
            # [u, u+1024) of the 1088-byte window (overlapping reads).
            mraw = pool.tile([NTAP, TJ], fp8, tag="mraw")
            mask_src = AP(maskr_d[:].tensor, 0, [(1, NTAP), (1, TJ)])
            nc.sync.dma_start(mraw[:], mask_src)
            # S4 sparse stationary (cols 0:128) + noise rows (cols 128:384)
            nm = pool.tile([4, 384], bf16, tag="nm")
            nc.sync.dma_start(nm[:], nm_d[:])
            s4 = nm[:, 0:128]
            noisem = nm[:, 128:384]

            wrow = crit[0:1, 0:128]
            srow = crit[0:1, 128:256]
            onef = nc.const_aps.tensor(1.0, (1, 1), f32)
            oneb = nc.const_aps.tensor(1.0, (1, 1), bf16)

            # ---- input-independent prep (Pool engine, off-path) ----
            # d2n[., u] = -(u-31)^2 via (u-31)*(31-u)
            dlt = pool.tile([128, NTAP], f32, tag="dlt")
            nc.gpsimd.iota(
                dlt[:], pattern=[[1, NTAP]], base=-31, channel_multiplier=0,
                allow_small_or_imprecise_dtypes=True,
            )
            dltn = pool.tile([128, NTAP], f32, tag="dltn")
            nc.gpsimd.iota(
                dltn[:], pattern=[[-1, NTAP]], base=31, channel_multiplier=0,
                allow_small_or_imprecise_dtypes=True,
            )
            d2n = pool.tile([128, NTAP], f32, tag="d2n")
            nc.gpsimd.tensor_mul(d2n[:], dlt[:], dltn[:])

            # ---- transpose sigma early: row -> column (PE, f32) ----
            pSc = psum.tile([128, 1], f32, tag="pSc")
            nc.tensor.matmul(pSc[:], srow, onef, is_transpose=True)
            sc = pool.tile([128, 1], f32, tag="sc")
            nc.vector.tensor_copy(sc[:], pSc[:])
            # column-space sigma chain (per-partition scalars, fast DVE)
            s2c = pool.tile([128, 1], f32, tag="s2c")
            nc.vector.tensor_scalar(
                s2c[:], sc[:], sc[:], 2.0, mybir.AluOpType.mult,
                mybir.AluOpType.mult,
            )
            ivc = pool.tile([128, 1], f32, tag="ivc")
            nc.vector.reciprocal(ivc[:], s2c[:])
            rsc = pool.tile([128, 1], f32, tag="rsc")
            nc.vector.reciprocal(rsc[:], sc[:])

            # ---- softmax pieces in row space (partition 0) ----
            # exp(w) with free-dim accumulate -> Z; then t1h = c*exp(w)/Z
            ew = pool.tile([1, 132], f32, tag="ew")
            nc.scalar.activation(
                ew[:, 0:128], wrow, mybir.ActivationFunctionType.Exp,
                accum_out=ew[:, 128:129],
            )
            czr = pool.tile([1, 1], f32, tag="czr")
            nc.vector.reciprocal(czr[:], ew[:, 128:129])
            t1h = pool.tile([1, 128], bf16, tag="t1h")
            nc.vector.tensor_scalar(
                t1h[:], ew[:, 0:128], czr[:], INV_SQRT_2PI,
                mybir.AluOpType.mult, mybir.AluOpType.mult,
            )
            # transpose t1h (bf16) and finish t1 = t1h / sigma in columns
            pTh = psum.tile([128, 1], bf16, tag="pTh")
            nc.tensor.matmul(pTh[:], t1h[:], oneb, is_transpose=True)
            t1c = pool.tile([128, 1], bf16, tag="t1c")
            nc.vector.tensor_mul(t1c[:], pTh[:], rsc[:])

            # ---- per-sigma exp table and combined kernel gs ----
            expt = pool.tile([128, NTAP], bf16, tag="expt")
            nc.scalar.activation(
                expt[:], d2n[:], mybir.ActivationFunctionType.Exp, scale=ivc[:]
            )
            gp = psum.tile([64, 1], f32, tag="gp")
            nc.tensor.matmul(gp[:], expt[:], t1c[:], start=True, stop=True)
            gs = pool.tile([64, 1], bf16, tag="gs")
            nc.vector.tensor_copy(gs[:], gp[:])

            # ---- binarize the Toeplitz mask (X-0.5 > 0), 2 DVE chunks ----
            mb = pool.tile([NTAP, TJ], bf16, tag="mb")
            for lo, hi in ((0, 512), (512, TJ)):
                nc.vector.tensor_scalar(
                    mb[:, lo:hi], mraw[:, lo:hi], 0.0, None, mybir.AluOpType.is_gt
                )

            # ---- output PSUM: noise via sparse-stationary matmul, then
            #      4 concurrent col-tiled conv matmuls ----
            po = psum.tile([128, 256], f32, tag="po")
            # start+stop on the noise matmul opens and closes the sim's
            # group tracking in one go; the partial-partition conv matmuls
            # accumulate with the bank-granular group check bypassed
            # (stop is sim-only; HW accumulation is per-element has_written).
            nc.tensor.matmul(po[:], s4[:], noisem[:], start=True, stop=True)
            for k in range(4):
                nc.tensor.matmul(
                    po[32 * k : 32 * k + 1, :], gs[:], mb[:, 256 * k : 256 * (k + 1)],
                    start=False, stop=False,
                    tile_position=(0, 32 * k),
                    skip_group_check=True,
                )

            # ---- store: copy live PSUM partitions out, 4 descriptors ----
            outS = pool.tile([128, 256], f32, tag="outS")
            nc.vector.tensor_copy(outS[:], po[:])
            out_src = AP(outS[:].tensor, outS[:].offset, [(32 * 256, 4), (1, 256)])
            nc.sync.dma_start(out_d[:], out_src)

    nc.compile()
    return nc


def kernel(X, sigma, weight, noise):
    global _compiled
    import ml_dtypes

    from concourse.bass_utils import run_bass_kernel_spmd

    X = np.ascontiguousarray(np.asarray(X, dtype=np.float32)).reshape(1, T)
    sigma = np.ascontiguousarray(np.asarray(sigma, dtype=np.float32)).reshape(E)
    weight = np.ascontiguousarray(np.asarray(weight, dtype=np.float32)).reshape(1, E)
    noise = np.ascontiguousarray(np.asarray(noise, dtype=np.float32)).reshape(1, T)

    if _compiled is None:
        _compiled = _build()
    nc = _compiled

    # mask window: 64*(X-0.5) as fp8 (sign-preserving affine shift; the
    # device binarizes with >0).  Window for core c covers global indices
    # [c*1024 - 32, c*1024 + 1055]; out-of-range pads to -32 (mask 0).
    Xpad = np.full(T + NTAP, -32.0, dtype=np.float32)
    Xpad[NTAP // 2 : NTAP // 2 + T] = 64.0 * (X[0] - 0.5)
    in_maps = []
    for c in range(N_CORES):
        crit = np.empty((1, 256), dtype=np.float32)
        crit[0, 0:128] = weight[0]
        crit[0, 128:256] = sigma
        maskr = Xpad[c * TJ : c * TJ + WIN].astype(ml_dtypes.float8_e4m3)
        nm = np.zeros((4, 384), dtype=ml_dtypes.bfloat16)
        for k in range(4):
            nm[k, 32 * k] = 1.0
        nm[:, 128:384] = (
            noise[0, c * TJ : (c + 1) * TJ].reshape(4, 256).astype(ml_dtypes.bfloat16)
        )
        in_maps.append({"crit": crit, "maskr": maskr.reshape(1, WIN), "nm": nm})

    res = run_bass_kernel_spmd(nc, in_maps, core_ids=list(range(N_CORES)))
    out = np.empty((1, T), dtype=np.float32)
    for c in range(N_CORES):
        out[0, c * TJ : (c + 1) * TJ] = res.results[c]["out"].reshape(-1)
    return out
